# revision 53
# baseline (speedup 1.0000x reference)
"""Bass/Trainium2 kernel for DisableGateLSTM (embedding -> LSTM -> max-pool -> FC).

Strategy: data-parallel over batch across 8 cores (8 rows/core, weights
replicated). Per core:
  Phase A: indirect-DMA gather of embedding rows + dense matmul precompute of
           xw_t = x_t @ Wx^T + b for all timesteps -> DRAM chunks.
  Phase B: the sequential recurrence. Layout: batch on PSUM partitions
           (rows 0..8), gates side by side on the free dim, so every
           elementwise operand shares partition range 0:8. Weight columns are
           host-reordered to (hidden-chunk, gate, hidden) so each of the
           three hidden-chunks [256, 128, 128] is a contiguous PSUM tile;
           the big chunk's ScalarE/VectorE chain overlaps the later chunks'
           PE matmuls, and the small tail chunks keep the step-to-step
           serial chain short. h^T staging is split into two PSUM tiles so
           the next step's matmul regions start as soon as their own
           K-tiles' copies land. Matmuls run as float32r (1 cycle/row vs 4
           for fp32; weights/activations are rounded by the PE, c/h state
           and PSUM accumulation stay fp32 — L2 rel err ~1.3e-4).
"""

import sys

sys.path.insert(0, "/opt/trn_rl_repo")

import numpy as np

VOCAB, EMBED, HIDDEN, CLASSES = 32000, 256, 512, 4
BATCH, SEQ = 64, 512
NCORES = 8
BPC = BATCH // NCORES  # batch rows per core
G = 4 * HIDDEN  # stacked gate width
P = 128
KT = HIDDEN // P  # 4 K-tiles for the recurrent contraction
CH = 3  # hidden chunks in phase B
CHWS = [256, 128, 128]  # chunk widths
CHST = [0, 256, 384]  # chunk start offsets in hidden
CGWS = [4 * w for w in CHWS]  # psum cols per chunk
CGST = [4 * st for st in CHST]  # chunk start offsets in gate-stacked cols

_nc_cache = {}


def build_nc(T=SEQ, f32r=True):
    import concourse.bass as bass
    import concourse.mybir as mybir
    from concourse import bacc
    import concourse.tile as tile
    from concourse.bass import ds, ts
    from concourse.masks import make_identity

    f32 = mybir.dt.float32
    i32 = mybir.dt.int32
    mmdt = mybir.dt.float32r if f32r else f32
    SIG = mybir.ActivationFunctionType.Sigmoid
    TANH = mybir.ActivationFunctionType.Tanh
    MUL = mybir.AluOpType.mult
    ADD = mybir.AluOpType.add
    MAX = mybir.AluOpType.max


    NCHUNK = (T * BPC + P - 1) // P
    TPC = P // BPC  # timesteps per phase-A chunk (16)

    nc = bacc.Bacc("TRN2", debug=False)
    ids_d = nc.dram_tensor("input_ids", [T * BPC, 1], i32, kind="ExternalInput")
    emb_d = nc.dram_tensor("embedding", [VOCAB, EMBED], f32, kind="ExternalInput")
    whT_d = nc.dram_tensor("whT", [HIDDEN, G], mmdt, kind="ExternalInput")
    wxT_d = nc.dram_tensor("wxT", [EMBED, G], mmdt, kind="ExternalInput")
    bias_d = nc.dram_tensor("bias", [1, G], mmdt, kind="ExternalInput")
    fcwT_d = nc.dram_tensor("fcwT", [HIDDEN, CLASSES], mmdt, kind="ExternalInput")
    fcb_d = nc.dram_tensor("fcb", [1, CLASSES], mmdt, kind="ExternalInput")
    out_d = nc.dram_tensor("logits", [BPC, CLASSES], f32, kind="ExternalOutput")

    with tile.TileContext(nc) as tc:
        with (
            tc.tile_pool(name="const", bufs=1) as constp,
            tc.tile_pool(name="wpool", bufs=1) as wp,
            tc.tile_pool(name="xwdram", bufs=1, space="DRAM") as dramp,
            tc.tile_pool(name="state", bufs=1) as statep,
        ):
            ident = constp.tile([P, P], f32, tag="ident")
            make_identity(nc, ident[:])
            ones1 = constp.tile([1, P], f32, tag="ones1")
            nc.gpsimd.memset(ones1[:], 1.0)
            identr = constp.tile([P, P], mmdt, tag="identr")
            nc.vector.tensor_copy(identr[:], ident[:])
            onesr = constp.tile([1, P], mmdt, tag="onesr")
            nc.vector.tensor_copy(onesr[:], ones1[:])

            whT_sb = wp.tile([P, KT, G], mmdt, tag="whT")
            nc.sync.dma_start(
                whT_sb[:], whT_d[:].rearrange("(kt p) n -> p kt n", p=P)
            )
            bias_sb = wp.tile([1, G], mmdt, tag="bias")
            nc.sync.dma_start(bias_sb[:], bias_d[:])
            fcwT_sb = wp.tile([P, KT, CLASSES], mmdt, tag="fcwT")
            nc.sync.dma_start(
                fcwT_sb[:], fcwT_d[:].rearrange("(kt p) c -> p kt c", p=P)
            )
            fcb_sb = wp.tile([1, CLASSES], mmdt, tag="fcb")
            nc.sync.dma_start(fcb_sb[:], fcb_d[:])

            xw_ch = [
                dramp.tile([P, G], mmdt, tag=f"xw{m}", name=f"xw{m}")
                for m in range(NCHUNK)
            ]

            # ---------------- Phase A: xw_t = x_t @ Wx^T + b ----------------
            with (
                tc.tile_pool(name="pA", bufs=3) as pa,
                tc.tile_pool(name="pAw", bufs=1) as paw,
                tc.tile_pool(name="pAps", bufs=2, space="PSUM") as paps,
            ):
                wxT_sb = paw.tile([P, 2, G], mmdt, tag="wxT")
                nc.sync.dma_start(
                    wxT_sb[:], wxT_d[:].rearrange("(kt p) n -> p kt n", p=P)
                )
                for m in range(NCHUNK):
                    ids_sb = pa.tile([P, 1], i32, tag="ids")
                    nc.sync.dma_start(ids_sb[:], ids_d[ts(m, P), :])
                    x_sb = pa.tile([P, EMBED], f32, tag="x")
                    nc.gpsimd.indirect_dma_start(
                        out=x_sb[:],
                        out_offset=None,
                        in_=emb_d[:],
                        in_offset=bass.IndirectOffsetOnAxis(
                            ap=ids_sb[:, :1], axis=0
                        ),
                    )
                    xT_ps = paps.tile([P, 2, P], f32, tag="xT")
                    for q in range(2):
                        nc.tensor.transpose(
                            xT_ps[:, q, :], x_sb[:, ts(q, P)], ident[:]
                        )
                    xT_sb = pa.tile([P, 2, P], mmdt, tag="xTs")
                    nc.vector.tensor_copy(xT_sb[:], xT_ps[:])
                    for half in range(2):
                        xw_ps = paps.tile(
                            [P, 1024], f32, tag=f"xwps{half}",
                            name=f"xwps{half}", bufs=1,
                        )
                        for n in range(2):
                            sl = ds(1024 * half + 512 * n, 512)
                            nc.tensor.matmul(
                                xw_ps[:, ts(n, 512)],
                                onesr[:1, :P],
                                bias_sb[:1, sl],
                                start=True,
                                stop=False,
                            )
                            for q in range(2):
                                nc.tensor.matmul(
                                    xw_ps[:, ts(n, 512)],
                                    xT_sb[:, q, :],
                                    wxT_sb[:, q, sl],
                                    start=False,
                                    stop=(q == 1),
                                )
                        stage = pa.tile([P, 1024], mmdt, tag=f"stage{half}")
                        if half == 0:
                            nc.vector.tensor_copy(stage[:], xw_ps[:])
                        else:
                            nc.scalar.copy(stage[:], xw_ps[:])
                        nc.sync.dma_start(
                            xw_ch[m][:, ds(1024 * half, 1024)], stage[:]
                        )

            # ---------------- Phase B: the recurrence ----------------
            cst = [
                statep.tile([BPC, HIDDEN], f32, tag=f"c{i}", name=f"c{i}")
                for i in range(2)
            ]
            hTt = [
                statep.tile([P, KT * BPC], mmdt, tag=f"hT{i}", name=f"hT{i}")
                for i in range(2)
            ]
            hm8 = statep.tile([BPC, HIDDEN], f32, tag="hm8")

            with (
                tc.tile_pool(name="pB", bufs=3) as pb,
                tc.tile_pool(name="xrp", bufs=6) as xrp,
                tc.tile_pool(name="pBps", bufs=1, space="PSUM") as pbps,
                tc.tile_pool(name="pBps2", bufs=2, space="PSUM") as pbps2,
            ):
                for t in range(T):
                    m, rr = divmod(t, TPC)
                    xr = xrp.tile([BPC, G], mmdt, tag="xr")
                    nc.sync.dma_start(xr[:], xw_ch[m][ds(BPC * rr, BPC), :])
                    # chunk-c psum: [f i o g] for hidden slice c.
                    # chunk 0 is read early in the step so WAR rarely binds;
                    # the small tail chunk is read late: double-buffer it.
                    psc = [
                        pbps.tile(
                            [BPC, CGWS[c]], f32, tag=f"ps{c}", name=f"ps{c}",
                            bufs=(1 if c == 0 else 2),
                        )
                        for c in range(CH)
                    ]

                    sig = pb.tile([BPC, 3 * HIDDEN], f32, tag="sig")
                    gt = pb.tile([BPC, HIDDEN], f32, tag="g")
                    tct = pb.tile([BPC, HIDDEN], f32, tag="tc")
                    m1 = pb.tile([BPC, HIDDEN], f32, tag="m1")
                    m2 = pb.tile([BPC, HIDDEN], f32, tag="m2")
                    ht = pb.tile([BPC, HIDDEN], f32, tag="h")
                    for c in range(CH):
                        cs = ds(CHST[c], CHWS[c])
                        for off in range(0, CGWS[c], 512):
                            w = min(512, CGWS[c] - off)
                            nc.tensor.matmul(
                                psc[c][:, ds(off, w)],
                                identr[:BPC, :BPC],
                                xr[:BPC, ds(CGST[c] + off, w)],
                                start=True,
                                stop=(t == 0),
                            )
                            if t > 0:
                                for k in range(KT):
                                    nc.tensor.matmul(
                                        psc[c][:, ds(off, w)],
                                        hTt[(t - 1) % 2][:, ts(k, BPC)],
                                        whT_sb[:, k, ds(CGST[c] + off, w)],
                                        start=False,
                                        stop=(k == KT - 1),
                                    )
                        # sigmoid over [f i o], tanh over g — all at rows 0:8
                        nc.scalar.activation(
                            sig[:, ds(3 * CHST[c], 3 * CHWS[c])],
                            psc[c][:, 0 : 3 * CHWS[c]],
                            SIG,
                        )
                        nc.scalar.activation(
                            gt[:, cs], psc[c][:, 3 * CHWS[c] : 4 * CHWS[c]],
                            TANH,
                        )
                        if t > 0:
                            nc.vector.tensor_tensor(
                                m1[:, cs],
                                sig[:, ds(3 * CHST[c], CHWS[c])],
                                cst[(t - 1) % 2][:, cs],
                                op=MUL,
                            )
                            nc.vector.tensor_tensor(
                                m2[:, cs],
                                sig[:, ds(3 * CHST[c] + CHWS[c], CHWS[c])],
                                gt[:, cs],
                                op=MUL,
                            )
                            nc.vector.tensor_tensor(
                                cst[t % 2][:, cs], m1[:, cs], m2[:, cs], op=ADD
                            )
                        else:
                            nc.gpsimd.tensor_tensor(
                                cst[0][:, cs],
                                sig[:, ds(3 * CHST[c] + CHWS[c], CHWS[c])],
                                gt[:, cs],
                                op=MUL,
                            )
                        nc.scalar.activation(
                            tct[:, cs], cst[t % 2][:, cs], TANH
                        )
                        nc.vector.tensor_tensor(
                            ht[:, cs],
                            sig[:, ds(3 * CHST[c] + 2 * CHWS[c], CHWS[c])],
                            tct[:, cs],
                            op=MUL,
                        )
                    # h^T K-tiles for next step's stationary operand.
                    # Two psum tiles so the k3 copy (critical loop) only
                    # waits its own transpose, not all four.
                    hT_pa = pbps2.tile(
                        [P, 3 * BPC], f32, tag="hTpa", bufs=1
                    )
                    hT_pb = pbps2.tile([P, BPC], f32, tag="hTpb", bufs=1)
                    for k in range(3):
                        nc.tensor.transpose(
                            hT_pa[:, ts(k, BPC)],
                            ht[:, ts(k, P)],
                            ident[:BPC, :BPC],
                        )
                    nc.tensor.transpose(
                        hT_pb[:], ht[:, ts(3, P)], ident[:BPC, :BPC]
                    )
                    nc.scalar.copy(
                        hTt[t % 2][:, 0 : 3 * BPC], hT_pa[:]
                    )
                    nc.scalar.copy(
                        hTt[t % 2][:, 3 * BPC : 4 * BPC], hT_pb[:]
                    )
                    if t == 0:
                        nc.gpsimd.tensor_copy(hm8[:], ht[:])
                    else:
                        nc.vector.tensor_tensor(hm8[:], hm8[:], ht[:], op=MAX)

                # ---------------- finale: logits ----------------
                hmT_pa = pbps2.tile([P, 3 * BPC], f32, tag="hTpa", bufs=1)
                hmT_pb = pbps2.tile([P, BPC], f32, tag="hTpb", bufs=1)
                for k in range(3):
                    nc.tensor.transpose(
                        hmT_pa[:, ts(k, BPC)],
                        hm8[:, ts(k, P)],
                        ident[:BPC, :BPC],
                    )
                nc.tensor.transpose(
                    hmT_pb[:], hm8[:, ts(3, P)], ident[:BPC, :BPC]
                )
                hmT = pb.tile([P, KT * BPC], mmdt, tag="hmT")
                nc.vector.tensor_copy(hmT[:, 0 : 3 * BPC], hmT_pa[:])
                nc.vector.tensor_copy(hmT[:, 3 * BPC : 4 * BPC], hmT_pb[:])
                lg_ps = pbps2.tile([BPC, CLASSES], f32, tag="hTpa", bufs=1)
                nc.tensor.matmul(
                    lg_ps[:],
                    onesr[:1, :BPC],
                    fcb_sb[:1, :],
                    start=True,
                    stop=False,
                )
                for k in range(KT):
                    nc.tensor.matmul(
                        lg_ps[:],
                        hmT[:, ts(k, BPC)],
                        fcwT_sb[:, k, :],
                        start=False,
                        stop=(k == KT - 1),
                    )
                lg_sb = pb.tile([BPC, CLASSES], f32, tag="lgsb")
                nc.vector.tensor_copy(lg_sb[:], lg_ps[:])
                nc.sync.dma_start(out_d[:], lg_sb[:])

    nc.compile()
    return nc


def _reorder_cols(w):
    """[*, 4*H] gate-stacked -> (chunk, gate, hidden-within-chunk) order."""
    gates = [w[:, j * HIDDEN : (j + 1) * HIDDEN] for j in range(4)]
    parts = []
    for c in range(CH):
        for gv in gates:
            parts.append(gv[:, CHST[c] : CHST[c] + CHWS[c]])
    return np.ascontiguousarray(np.concatenate(parts, axis=1))


def _hilo(w, bits=8):
    """Split fp32 into a 10-explicit-mantissa-bit hi part (exact in f32r)
    plus the residual; hi + lo reconstructs w to ~2^-21 relative."""
    u = w.view(np.uint32)
    hi = ((u >> (23 - bits)) << (23 - bits)).view(np.float32)
    lo = (w - hi).astype(np.float32)
    return np.ascontiguousarray(np.concatenate([hi, lo], axis=1))


def prep_host_inputs(inputs, T=SEQ):
    ids = np.ascontiguousarray(np.asarray(inputs["input_ids"]).astype(np.int32))
    emb = np.ascontiguousarray(np.asarray(inputs["embedding"], dtype=np.float32))
    Ws = [np.asarray(inputs[f"W_{g}"], dtype=np.float32) for g in "fioc"]
    bs = [np.asarray(inputs[f"b_{g}"], dtype=np.float32) for g in "fioc"]
    whT = _reorder_cols(np.concatenate([W[:, :HIDDEN].T for W in Ws], axis=1))
    wxT = _reorder_cols(np.concatenate([W[:, HIDDEN:].T for W in Ws], axis=1))
    bias = _reorder_cols(np.concatenate(bs)[None, :])
    fcwT = np.ascontiguousarray(np.asarray(inputs["fc_w"], dtype=np.float32).T)
    fcb = np.ascontiguousarray(
        np.asarray(inputs["fc_b"], dtype=np.float32)[None, :]
    )
    in_maps = []
    for c in range(NCORES):
        in_maps.append(
            {
                "input_ids": np.ascontiguousarray(
                    ids[c * BPC : (c + 1) * BPC, :T].T.reshape(T * BPC, 1)
                ),
                "embedding": emb,
                "whT": whT,
                "wxT": wxT,
                "bias": bias,
                "fcwT": fcwT,
                "fcb": fcb,
            }
        )
    return in_maps


def run(inputs, T=SEQ, trace=False, f32r=True):
    from concourse.bass_utils import run_bass_kernel_spmd

    key = (T, f32r)
    if key not in _nc_cache:
        _nc_cache[key] = build_nc(T, f32r=f32r)
    nc = _nc_cache[key]
    in_maps = prep_host_inputs(inputs, T)
    res = run_bass_kernel_spmd(nc, in_maps, list(range(NCORES)), trace=trace)
    out = np.concatenate([r["logits"] for r in res.results], axis=0)
    return out, res


def kernel(**inputs) -> np.ndarray:
    out, _ = run(inputs, T=SEQ, trace=False)
    return out


# revision 56
# speedup vs baseline: 1.4809x; 1.4809x over previous
"""Bass/Trainium2 kernel for DisableGateLSTM (embedding -> LSTM -> max-pool -> FC).

Strategy: data-parallel over batch across 8 cores (8 rows/core, weights
replicated). Per core:
  Phase A: indirect-DMA gather of embedding rows + dense matmul precompute of
           xw_t = x_t @ Wx^T + b for all timesteps -> DRAM chunks.
  Phase B: the sequential recurrence. Layout: batch on PSUM partitions
           (rows 0..8), gates side by side on the free dim, so every
           elementwise operand shares partition range 0:8. Weight columns are
           host-reordered to (hidden-chunk, gate, hidden) so each of the
           three hidden-chunks [256, 128, 128] is a contiguous PSUM tile;
           the big chunk's ScalarE/VectorE chain overlaps the later chunks'
           PE matmuls, and the small tail chunks keep the step-to-step
           serial chain short. h^T staging is split into two PSUM tiles so
           the next step's matmul regions start as soon as their own
           K-tiles' copies land. Matmuls run as float32r (1 cycle/row vs 4
           for fp32; weights/activations are rounded by the PE, c/h state
           and PSUM accumulation stay fp32 — L2 rel err ~1.3e-4).
"""

import sys

sys.path.insert(0, "/opt/trn_rl_repo")

import numpy as np

VOCAB, EMBED, HIDDEN, CLASSES = 32000, 256, 512, 4
BATCH, SEQ = 64, 512
NCORES = 8
BPC = BATCH // NCORES  # batch rows per core
G = 4 * HIDDEN  # stacked gate width
P = 128
KT = HIDDEN // P  # 4 K-tiles for the recurrent contraction
CH = 3  # hidden chunks in phase B
CHWS = [256, 128, 128]  # chunk widths
CHST = [0, 256, 384]  # chunk start offsets in hidden
CGWS = [4 * w for w in CHWS]  # psum cols per chunk
CGST = [4 * st for st in CHST]  # chunk start offsets in gate-stacked cols

_nc_cache = {}


def build_nc(T=SEQ, f32r=True):
    import concourse.bass as bass
    import concourse.mybir as mybir
    from concourse import bacc
    import concourse.tile as tile
    from concourse.bass import ds, ts
    from concourse.masks import make_identity

    f32 = mybir.dt.float32
    i32 = mybir.dt.int32
    mmdt = mybir.dt.float32r if f32r else f32
    SIG = mybir.ActivationFunctionType.Sigmoid
    TANH = mybir.ActivationFunctionType.Tanh
    MUL = mybir.AluOpType.mult
    ADD = mybir.AluOpType.add
    MAX = mybir.AluOpType.max


    NCHUNK = (T * BPC + P - 1) // P
    TPC = P // BPC  # timesteps per phase-A chunk (16)

    nc = bacc.Bacc("TRN2", debug=False)
    ids_d = nc.dram_tensor("input_ids", [T * BPC, 1], i32, kind="ExternalInput")
    emb_d = nc.dram_tensor("embedding", [VOCAB, EMBED], f32, kind="ExternalInput")
    whT_d = nc.dram_tensor("whT", [HIDDEN, G], mmdt, kind="ExternalInput")
    wxT_d = nc.dram_tensor("wxT", [EMBED, G], mmdt, kind="ExternalInput")
    bias_d = nc.dram_tensor("bias", [1, G], mmdt, kind="ExternalInput")
    fcwT_d = nc.dram_tensor("fcwT", [HIDDEN, CLASSES], mmdt, kind="ExternalInput")
    fcb_d = nc.dram_tensor("fcb", [1, CLASSES], mmdt, kind="ExternalInput")
    out_d = nc.dram_tensor("logits", [BPC, CLASSES], f32, kind="ExternalOutput")

    with tile.TileContext(nc) as tc:
        with (
            tc.tile_pool(name="const", bufs=1) as constp,
            tc.tile_pool(name="wpool", bufs=1) as wp,
            tc.tile_pool(name="xwdram", bufs=1, space="DRAM") as dramp,
            tc.tile_pool(name="state", bufs=1) as statep,
        ):
            ident = constp.tile([P, P], f32, tag="ident")
            make_identity(nc, ident[:])
            ones1 = constp.tile([1, P], f32, tag="ones1")
            nc.gpsimd.memset(ones1[:], 1.0)
            identr = constp.tile([P, P], mmdt, tag="identr")
            nc.vector.tensor_copy(identr[:], ident[:])
            onesr = constp.tile([1, P], mmdt, tag="onesr")
            nc.vector.tensor_copy(onesr[:], ones1[:])

            whT_sb = wp.tile([P, KT, G], mmdt, tag="whT")
            nc.sync.dma_start(
                whT_sb[:], whT_d[:].rearrange("(kt p) n -> p kt n", p=P)
            )
            bias_sb = wp.tile([1, G], mmdt, tag="bias")
            nc.sync.dma_start(bias_sb[:], bias_d[:])
            fcwT_sb = wp.tile([P, KT, CLASSES], mmdt, tag="fcwT")
            nc.sync.dma_start(
                fcwT_sb[:], fcwT_d[:].rearrange("(kt p) c -> p kt c", p=P)
            )
            fcb_sb = wp.tile([1, CLASSES], mmdt, tag="fcb")
            nc.sync.dma_start(fcb_sb[:], fcb_d[:])

            xw_ch = [
                dramp.tile([P, G], mmdt, tag=f"xw{m}", name=f"xw{m}")
                for m in range(NCHUNK)
            ]

            # ---------------- Phase A: xw_t = x_t @ Wx^T + b ----------------
            with (
                tc.tile_pool(name="pA", bufs=3) as pa,
                tc.tile_pool(name="pAw", bufs=1) as paw,
                tc.tile_pool(name="pAps", bufs=2, space="PSUM") as paps,
            ):
                wxT_sb = paw.tile([P, 2, G], mmdt, tag="wxT")
                nc.sync.dma_start(
                    wxT_sb[:], wxT_d[:].rearrange("(kt p) n -> p kt n", p=P)
                )
                for m in range(NCHUNK):
                    ids_sb = pa.tile([P, 1], i32, tag="ids")
                    nc.sync.dma_start(ids_sb[:], ids_d[ts(m, P), :])
                    x_sb = pa.tile([P, EMBED], f32, tag="x")
                    nc.gpsimd.indirect_dma_start(
                        out=x_sb[:],
                        out_offset=None,
                        in_=emb_d[:],
                        in_offset=bass.IndirectOffsetOnAxis(
                            ap=ids_sb[:, :1], axis=0
                        ),
                    )
                    xT_ps = paps.tile([P, 2, P], f32, tag="xT")
                    for q in range(2):
                        nc.tensor.transpose(
                            xT_ps[:, q, :], x_sb[:, ts(q, P)], ident[:]
                        )
                    xT_sb = pa.tile([P, 2, P], mmdt, tag="xTs")
                    nc.vector.tensor_copy(xT_sb[:], xT_ps[:])
                    for half in range(2):
                        xw_ps = paps.tile(
                            [P, 1024], f32, tag=f"xwps{half}",
                            name=f"xwps{half}", bufs=1,
                        )
                        for n in range(2):
                            sl = ds(1024 * half + 512 * n, 512)
                            nc.tensor.matmul(
                                xw_ps[:, ts(n, 512)],
                                onesr[:1, :P],
                                bias_sb[:1, sl],
                                start=True,
                                stop=False,
                            )
                            for q in range(2):
                                nc.tensor.matmul(
                                    xw_ps[:, ts(n, 512)],
                                    xT_sb[:, q, :],
                                    wxT_sb[:, q, sl],
                                    start=False,
                                    stop=(q == 1),
                                )
                        stage = pa.tile([P, 1024], mmdt, tag=f"stage{half}")
                        if half == 0:
                            nc.vector.tensor_copy(stage[:], xw_ps[:])
                        else:
                            nc.scalar.copy(stage[:], xw_ps[:])
                        nc.sync.dma_start(
                            xw_ch[m][:, ds(1024 * half, 1024)], stage[:]
                        )

            # ---------------- Phase B: the recurrence ----------------
            cst = [
                statep.tile([BPC, HIDDEN], f32, tag=f"c{i}", name=f"c{i}")
                for i in range(2)
            ]
            hTt = [
                statep.tile([P, KT * BPC], mmdt, tag=f"hT{i}", name=f"hT{i}")
                for i in range(2)
            ]
            hm8 = statep.tile([BPC, HIDDEN], f32, tag="hm8")

            with (
                tc.tile_pool(name="pB", bufs=3) as pb,
                tc.tile_pool(name="xrp", bufs=6) as xrp,
                tc.tile_pool(name="pBps", bufs=1, space="PSUM") as pbps,
                tc.tile_pool(name="pBps2", bufs=2, space="PSUM") as pbps2,
            ):
                for t in range(T):
                    m, rr = divmod(t, TPC)
                    xr = xrp.tile([BPC, G], mmdt, tag="xr")
                    nc.sync.dma_start(xr[:], xw_ch[m][ds(BPC * rr, BPC), :])
                    # chunk-c psum: [f i o g] for hidden slice c.
                    # chunk 0 is read early in the step so WAR rarely binds;
                    # the small tail chunk is read late: double-buffer it.
                    psc = [
                        pbps.tile(
                            [BPC, CGWS[c]], f32, tag=f"ps{c}", name=f"ps{c}",
                            bufs=(2 if c == 1 else 1),
                        )
                        for c in range(CH)
                    ]

                    sig = pb.tile([BPC, 3 * HIDDEN], f32, tag="sig")
                    gt = pb.tile([BPC, HIDDEN], f32, tag="g")
                    tct = pb.tile([BPC, HIDDEN], f32, tag="tc")
                    m1 = pb.tile([BPC, HIDDEN], f32, tag="m1")
                    m2 = pb.tile([BPC, HIDDEN], f32, tag="m2")
                    ht = pb.tile([BPC, HIDDEN], f32, tag="h")
                    for c in range(CH):
                        cs = ds(CHST[c], CHWS[c])
                        for off in range(0, CGWS[c], 512):
                            w = min(512, CGWS[c] - off)
                            nc.tensor.matmul(
                                psc[c][:, ds(off, w)],
                                identr[:BPC, :BPC],
                                xr[:BPC, ds(CGST[c] + off, w)],
                                start=True,
                                stop=(t == 0),
                            )
                            if t > 0:
                                for k in range(KT):
                                    nc.tensor.matmul(
                                        psc[c][:, ds(off, w)],
                                        hTt[(t - 1) % 2][:, ts(k, BPC)],
                                        whT_sb[:, k, ds(CGST[c] + off, w)],
                                        start=False,
                                        stop=(k == KT - 1),
                                    )
                        # sigmoid over [f i o], tanh over g — all at rows 0:8
                        nc.scalar.activation(
                            sig[:, ds(3 * CHST[c], 3 * CHWS[c])],
                            psc[c][:, 0 : 3 * CHWS[c]],
                            SIG,
                        )
                        nc.scalar.activation(
                            gt[:, cs], psc[c][:, 3 * CHWS[c] : 4 * CHWS[c]],
                            TANH,
                        )
                        if t > 0:
                            nc.vector.tensor_tensor(
                                m1[:, cs],
                                sig[:, ds(3 * CHST[c], CHWS[c])],
                                cst[(t - 1) % 2][:, cs],
                                op=MUL,
                            )
                            nc.vector.tensor_tensor(
                                m2[:, cs],
                                sig[:, ds(3 * CHST[c] + CHWS[c], CHWS[c])],
                                gt[:, cs],
                                op=MUL,
                            )
                            nc.vector.tensor_tensor(
                                cst[t % 2][:, cs], m1[:, cs], m2[:, cs], op=ADD
                            )
                        else:
                            nc.gpsimd.tensor_tensor(
                                cst[0][:, cs],
                                sig[:, ds(3 * CHST[c] + CHWS[c], CHWS[c])],
                                gt[:, cs],
                                op=MUL,
                            )
                        nc.scalar.activation(
                            tct[:, cs], cst[t % 2][:, cs], TANH
                        )
                        nc.vector.tensor_tensor(
                            ht[:, cs],
                            sig[:, ds(3 * CHST[c] + 2 * CHWS[c], CHWS[c])],
                            tct[:, cs],
                            op=MUL,
                        )
                    # h^T K-tiles for next step's stationary operand.
                    # Two psum tiles so the k3 copy (critical loop) only
                    # waits its own transpose, not all four.
                    hT_pa = pbps2.tile(
                        [P, 2 * BPC], f32, tag="hTpa", bufs=1
                    )
                    hT_pm = pbps2.tile([P, BPC], f32, tag="hTpm", bufs=1)
                    hT_pb = pbps2.tile([P, BPC], f32, tag="hTpb", bufs=1)
                    for k in range(2):
                        nc.tensor.transpose(
                            hT_pa[:, ts(k, BPC)],
                            ht[:, ts(k, P)],
                            ident[:BPC, :BPC],
                        )
                    nc.tensor.transpose(
                        hT_pm[:], ht[:, ts(2, P)], ident[:BPC, :BPC]
                    )
                    nc.tensor.transpose(
                        hT_pb[:], ht[:, ts(3, P)], ident[:BPC, :BPC]
                    )
                    nc.scalar.copy(
                        hTt[t % 2][:, 0 : 2 * BPC], hT_pa[:]
                    )
                    nc.scalar.copy(
                        hTt[t % 2][:, 2 * BPC : 3 * BPC], hT_pm[:]
                    )
                    nc.scalar.copy(
                        hTt[t % 2][:, 3 * BPC : 4 * BPC], hT_pb[:]
                    )
                    if t == 0:
                        nc.gpsimd.tensor_copy(hm8[:], ht[:])
                    else:
                        nc.vector.tensor_tensor(hm8[:], hm8[:], ht[:], op=MAX)

                # ---------------- finale: logits ----------------
                hmT_pa = pbps2.tile([P, 2 * BPC], f32, tag="hTpa", bufs=1)
                hmT_pm = pbps2.tile([P, BPC], f32, tag="hTpm", bufs=1)
                hmT_pb = pbps2.tile([P, BPC], f32, tag="hTpb", bufs=1)
                for k in range(2):
                    nc.tensor.transpose(
                        hmT_pa[:, ts(k, BPC)],
                        hm8[:, ts(k, P)],
                        ident[:BPC, :BPC],
                    )
                nc.tensor.transpose(
                    hmT_pm[:], hm8[:, ts(2, P)], ident[:BPC, :BPC]
                )
                nc.tensor.transpose(
                    hmT_pb[:], hm8[:, ts(3, P)], ident[:BPC, :BPC]
                )
                hmT = pb.tile([P, KT * BPC], mmdt, tag="hmT")
                nc.vector.tensor_copy(hmT[:, 0 : 2 * BPC], hmT_pa[:])
                nc.vector.tensor_copy(hmT[:, 2 * BPC : 3 * BPC], hmT_pm[:])
                nc.vector.tensor_copy(hmT[:, 3 * BPC : 4 * BPC], hmT_pb[:])
                lg_ps = pbps2.tile([BPC, CLASSES], f32, tag="hTpa", bufs=1)
                nc.tensor.matmul(
                    lg_ps[:],
                    onesr[:1, :BPC],
                    fcb_sb[:1, :],
                    start=True,
                    stop=False,
                )
                for k in range(KT):
                    nc.tensor.matmul(
                        lg_ps[:],
                        hmT[:, ts(k, BPC)],
                        fcwT_sb[:, k, :],
                        start=False,
                        stop=(k == KT - 1),
                    )
                lg_sb = pb.tile([BPC, CLASSES], f32, tag="lgsb")
                nc.vector.tensor_copy(lg_sb[:], lg_ps[:])
                nc.sync.dma_start(out_d[:], lg_sb[:])

    nc.compile()
    return nc


def _reorder_cols(w):
    """[*, 4*H] gate-stacked -> (chunk, gate, hidden-within-chunk) order."""
    gates = [w[:, j * HIDDEN : (j + 1) * HIDDEN] for j in range(4)]
    parts = []
    for c in range(CH):
        for gv in gates:
            parts.append(gv[:, CHST[c] : CHST[c] + CHWS[c]])
    return np.ascontiguousarray(np.concatenate(parts, axis=1))


def _hilo(w, bits=8):
    """Split fp32 into a 10-explicit-mantissa-bit hi part (exact in f32r)
    plus the residual; hi + lo reconstructs w to ~2^-21 relative."""
    u = w.view(np.uint32)
    hi = ((u >> (23 - bits)) << (23 - bits)).view(np.float32)
    lo = (w - hi).astype(np.float32)
    return np.ascontiguousarray(np.concatenate([hi, lo], axis=1))


def prep_host_inputs(inputs, T=SEQ):
    ids = np.ascontiguousarray(np.asarray(inputs["input_ids"]).astype(np.int32))
    emb = np.ascontiguousarray(np.asarray(inputs["embedding"], dtype=np.float32))
    Ws = [np.asarray(inputs[f"W_{g}"], dtype=np.float32) for g in "fioc"]
    bs = [np.asarray(inputs[f"b_{g}"], dtype=np.float32) for g in "fioc"]
    whT = _reorder_cols(np.concatenate([W[:, :HIDDEN].T for W in Ws], axis=1))
    wxT = _reorder_cols(np.concatenate([W[:, HIDDEN:].T for W in Ws], axis=1))
    bias = _reorder_cols(np.concatenate(bs)[None, :])
    fcwT = np.ascontiguousarray(np.asarray(inputs["fc_w"], dtype=np.float32).T)
    fcb = np.ascontiguousarray(
        np.asarray(inputs["fc_b"], dtype=np.float32)[None, :]
    )
    in_maps = []
    for c in range(NCORES):
        in_maps.append(
            {
                "input_ids": np.ascontiguousarray(
                    ids[c * BPC : (c + 1) * BPC, :T].T.reshape(T * BPC, 1)
                ),
                "embedding": emb,
                "whT": whT,
                "wxT": wxT,
                "bias": bias,
                "fcwT": fcwT,
                "fcb": fcb,
            }
        )
    return in_maps


def run(inputs, T=SEQ, trace=False, f32r=True):
    from concourse.bass_utils import run_bass_kernel_spmd

    key = (T, f32r)
    if key not in _nc_cache:
        _nc_cache[key] = build_nc(T, f32r=f32r)
    nc = _nc_cache[key]
    in_maps = prep_host_inputs(inputs, T)
    res = run_bass_kernel_spmd(nc, in_maps, list(range(NCORES)), trace=trace)
    out = np.concatenate([r["logits"] for r in res.results], axis=0)
    return out, res


def kernel(**inputs) -> np.ndarray:
    out, _ = run(inputs, T=SEQ, trace=False)
    return out
